# revision 10
# baseline (speedup 1.0000x reference)
"""Trainium2 Bass kernel for nn_CrossAttention (B=4, T=1024, S=2048, D=1024, H=16).

Sharding: tensor-parallel over heads. Each of the 8 cores owns 2 heads
(a 128-column slice of the q/k/v projections and the matching 128-row
slice of the o-projection input). Every core computes a full-shape
partial o-proj output; the host sums the 8 partials (the all-reduce is
done on the host during the gather/unshard step).

Layout strategy: all device matmuls contract along the SBUF partition
axis, so the host pre-transposes the activations and weights (free on
host, removes every on-chip transpose):
  xT  [D, B*T]  = query^T          (bf16)
  kvT [D, B*S]  = key_value^T      (bf16)
  wqT/wkT/wvT [D, 128] per core    (bf16)
  woT [128, D] per core            (bf16)

Pipeline per core (all matmul accumulation in fp32 PSUM):
  qT = WqT.T @ xT            -> [128c, B*T]   (c on partitions)
  kT = WkT.T @ kvT           -> [128c, B*S]
  V  = kvT.T @ WvT           -> [s, c] tiles, stored ones-augmented [128, 65]
  scoresT = kT.T @ qT per head (K=64)         -> [128s, 512t] PSUM
  PT = exp(0.125 * scoresT)  (ACT, no max-subtraction: |scores| < ~7)
  attnT[h] += V_aug.T @ PT   -> [65, 512t]; row 64 = softmax rowsum (free)
  rowsum transposed to [t, 1] via K=1 matmuls, reciprocal on DVE
  o-proj per head (K=64, row-packed) -> [128t, dout] PSUM per head
  out = psum_h0 * (1/r0)[t] + psum_h1 * (1/r1)[t]   (per-partition scalars)
"""

import os
import sys
from contextlib import ExitStack

import numpy as np

for _p in (
    "/root/.axon_site",
    "/root/.axon_site/_ro/trn_rl_repo",
    "/root/.axon_site/_ro/pypackages",
    "/opt/trn_rl_repo",
):
    if os.path.isdir(_p) and _p not in sys.path:
        sys.path.append(_p)

import ml_dtypes  # noqa: E402

import concourse.bass as bass  # noqa: E402
import concourse.mybir as mybir  # noqa: E402
import concourse.tile as tile  # noqa: E402
from concourse import bacc  # noqa: E402

BF = mybir.dt.bfloat16
F32 = mybir.dt.float32
NPBF = ml_dtypes.bfloat16

B, T, S, D = 4, 1024, 2048, 1024
BT, BS = B * T, B * S
P = 128
NCORES = 8
KT = D // P          # 8 contraction tiles of 128
TCH = 512            # free-dim chunk for projections / attention t-chunks
NJ = S // P          # 16 s-tiles of 128 per batch
NST = BS // P        # 64 s-tiles total
EXP_SCALE = float(64 ** -0.5)  # folded into the ACT exp


def build_nc():
    nc = bacc.Bacc("TRN2", target_bir_lowering=False)

    xT = nc.dram_tensor("xT", [D, BT], BF, kind="ExternalInput").ap()
    kvT = nc.dram_tensor("kvT", [D, BS], BF, kind="ExternalInput").ap()
    wqT = nc.dram_tensor("wqT", [D, P], BF, kind="ExternalInput").ap()
    wkT = nc.dram_tensor("wkT", [D, P], BF, kind="ExternalInput").ap()
    wvT = nc.dram_tensor("wvT", [D, P], BF, kind="ExternalInput").ap()
    woT = nc.dram_tensor("woT", [P, D], BF, kind="ExternalInput").ap()
    out = nc.dram_tensor("out", [BT, D], F32, kind="ExternalOutput").ap()

    with tile.TileContext(nc) as tc, ExitStack() as ctx:
        consts = ctx.enter_context(tc.tile_pool(name="consts", bufs=1))
        big = ctx.enter_context(tc.tile_pool(name="big", bufs=1))
        xin = ctx.enter_context(tc.tile_pool(name="xin", bufs=3))
        ptp = ctx.enter_context(tc.tile_pool(name="ptp", bufs=3))
        atsb = ctx.enter_context(tc.tile_pool(name="atsb", bufs=2))
        smalls = ctx.enter_context(tc.tile_pool(name="smalls", bufs=4))
        outp = ctx.enter_context(tc.tile_pool(name="outp", bufs=3))
        # PSUM budget (8 banks): mm 3 + at 2 + op 2 + rt 1
        mm_ps = ctx.enter_context(tc.tile_pool(name="mm_ps", bufs=3, space="PSUM"))
        at_pool = ctx.enter_context(tc.tile_pool(name="at_ps", bufs=2, space="PSUM"))
        op_pool = ctx.enter_context(tc.tile_pool(name="op_ps", bufs=2, space="PSUM"))
        rt_pool = ctx.enter_context(tc.tile_pool(name="rt_ps", bufs=1, space="PSUM"))

        # ---- resident weights ----
        wq_s = consts.tile([P, KT, P], BF, tag="wq_s")
        wk_s = consts.tile([P, KT, P], BF, tag="wk_s")
        wv_s = consts.tile([P, KT, P], BF, tag="wv_s")
        wqT_t = wqT.rearrange("(kt p) c -> p kt c", p=P)
        wkT_t = wkT.rearrange("(kt p) c -> p kt c", p=P)
        wvT_t = wvT.rearrange("(kt p) c -> p kt c", p=P)
        for kt in range(KT):
            nc.sync.dma_start(wq_s[:, kt, :], wqT_t[:, kt, :])
            nc.sync.dma_start(wk_s[:, kt, :], wkT_t[:, kt, :])
            nc.sync.dma_start(wv_s[:, kt, :], wvT_t[:, kt, :])
        wo_s = consts.tile([P, D], BF, tag="wo_s")
        nc.sync.dma_start(wo_s[:], woT)
        ones1 = consts.tile([1, 1], F32, tag="ones1")
        nc.gpsimd.memset(ones1[:], 1.0)

        # ---- resident intermediates ----
        qT_s = big.tile([P, BT], BF, tag="qT_s")
        kT_s = big.tile([P, BS], BF, tag="kT_s")
        # Per-head V, ones-augmented: 64 s-tiles, each [128, 65] with col 64 == 1.0
        v_s = [
            big.tile([P, NST * 65], BF, tag=f"v{h}_s", name=f"v{h}_s")
            for h in range(2)
        ]
        for h in range(2):
            nc.gpsimd.memset(v_s[h][:], 1.0)

        # ---- phase B: projections ----
        xT_t = xT.rearrange("(kt p) t -> p kt t", p=P)
        for ch in range(BT // TCH):  # 8 chunks
            x_t = xin.tile([P, KT, TCH], BF, tag="in_t")
            for kt in range(KT):
                nc.sync.dma_start(
                    x_t[:, kt, :], xT_t[:, kt, ch * TCH:(ch + 1) * TCH]
                )
            ps = mm_ps.tile([P, TCH], F32, tag="mm")
            for kt in range(KT):
                nc.tensor.matmul(
                    ps[:], wq_s[:, kt, :], x_t[:, kt, :],
                    start=(kt == 0), stop=(kt == KT - 1),
                )
            nc.vector.tensor_copy(qT_s[:, ch * TCH:(ch + 1) * TCH], ps[:])

        kvT_t = kvT.rearrange("(kt p) s -> p kt s", p=P)
        for ch in range(BS // TCH):  # 16 chunks
            kv_t = xin.tile([P, KT, TCH], BF, tag="in_t")
            for kt in range(KT):
                nc.sync.dma_start(
                    kv_t[:, kt, :], kvT_t[:, kt, ch * TCH:(ch + 1) * TCH]
                )
            ps = mm_ps.tile([P, TCH], F32, tag="mm")
            for kt in range(KT):
                nc.tensor.matmul(
                    ps[:], wk_s[:, kt, :], kv_t[:, kt, :],
                    start=(kt == 0), stop=(kt == KT - 1),
                )
            nc.vector.tensor_copy(kT_s[:, ch * TCH:(ch + 1) * TCH], ps[:])
            # V projection: [s, c] orientation, 4 s-subtiles share one PSUM bank
            vps = mm_ps.tile([P, 4, P], F32, tag="mm")
            # start only on the bank's first matmul: start=True marks the
            # whole 2KB zero-region pending-zero, so later subtiles' first
            # writes overwrite (not accumulate) stale data automatically.
            for kt in range(KT):
                for sub in range(4):
                    nc.tensor.matmul(
                        vps[:, sub, :],
                        kv_t[:, kt, sub * P:(sub + 1) * P],
                        wv_s[:, kt, :],
                        start=(kt == 0 and sub == 0),
                        stop=(kt == KT - 1 and sub == 3),
                    )
            for sub in range(4):
                jg = ch * 4 + sub
                nc.vector.tensor_copy(
                    v_s[0][:, jg * 65:jg * 65 + 64], vps[:, sub, 0:64]
                )
                nc.vector.tensor_copy(
                    v_s[1][:, jg * 65:jg * 65 + 64], vps[:, sub, 64:128]
                )

        # ---- phase C: attention + o-proj, per (batch, 512-wide t chunk) ----
        for b in range(B):
            for t2 in range(T // TCH):  # 2
                t0 = b * T + t2 * TCH
                ats = [
                    at_pool.tile([65, TCH], F32, tag="at", name=f"at{h}")
                    for h in range(2)
                ]
                for j in range(NJ):
                    jg = b * NJ + j
                    for h in range(2):
                        hp = h * 64
                        sc = mm_ps.tile([P, TCH], F32, tag="mm")
                        nc.tensor.matmul(
                            sc[:],
                            kT_s[hp:hp + 64, b * S + j * P: b * S + (j + 1) * P],
                            qT_s[hp:hp + 64, t0:t0 + TCH],
                            start=True, stop=True,
                        )
                        pt = ptp.tile([P, TCH], BF, tag="pt")
                        nc.scalar.activation(
                            pt[:], sc[:],
                            mybir.ActivationFunctionType.Exp,
                            scale=EXP_SCALE,
                        )
                        nc.tensor.matmul(
                            ats[h][:],
                            v_s[h][:, jg * 65:(jg + 1) * 65],
                            pt[:],
                            start=(j == 0), stop=(j == NJ - 1),
                        )

                # rowsums -> [t, 1] layout via K=1 matmuls, then reciprocal
                rt_ps = rt_pool.tile([P, 8], F32, tag="rt")
                aT = atsb.tile([P, TCH], BF, tag="aT")
                for h in range(2):
                    r_sb = smalls.tile([1, TCH], F32, tag="rsb")
                    nc.vector.tensor_copy(r_sb[:], ats[h][64:65, :])
                    for sub in range(4):
                        nc.tensor.matmul(
                            rt_ps[:, h * 4 + sub:h * 4 + sub + 1],
                            r_sb[0:1, sub * P:(sub + 1) * P],
                            ones1[0:1, 0:1],
                            start=True, stop=True,
                        )
                    nc.vector.tensor_copy(aT[h * 64:(h + 1) * 64, :], ats[h][0:64, :])
                rt = smalls.tile([P, 8], F32, tag="rtr")
                nc.vector.reciprocal(rt[:], rt_ps[:])

                # o-proj: per head (K=64, row-packed pair), combine with 1/r
                for sub in range(4):
                    ot = outp.tile([P, D], F32, tag="ot")
                    for n in range(D // TCH):  # 2
                        op0 = op_pool.tile([P, TCH], F32, tag="op")
                        op1 = op_pool.tile([P, TCH], F32, tag="op")
                        nc.tensor.matmul(
                            op0[:],
                            aT[0:64, sub * P:(sub + 1) * P],
                            wo_s[0:64, n * TCH:(n + 1) * TCH],
                            start=True, stop=True,
                        )
                        nc.tensor.matmul(
                            op1[:],
                            aT[64:128, sub * P:(sub + 1) * P],
                            wo_s[64:128, n * TCH:(n + 1) * TCH],
                            start=True, stop=True,
                        )
                        tmp = smalls.tile([P, TCH], F32, tag="tmp")
                        nc.vector.tensor_scalar_mul(tmp[:], op1[:], rt[:, 4 + sub:5 + sub])
                        nc.vector.tensor_scalar_mul(
                            ot[:, n * TCH:(n + 1) * TCH], op0[:], rt[:, sub:sub + 1]
                        )
                        nc.vector.tensor_add(
                            ot[:, n * TCH:(n + 1) * TCH],
                            ot[:, n * TCH:(n + 1) * TCH],
                            tmp[:],
                        )
                    nc.sync.dma_start(out[t0 + sub * P:t0 + (sub + 1) * P, :], ot[:])

    nc.compile()
    return nc


_NC_CACHE = None


def _get_nc():
    global _NC_CACHE
    if _NC_CACHE is None:
        _NC_CACHE = build_nc()
    return _NC_CACHE


def make_in_maps(query, key_value, wq, wk, wv, wo):
    q2 = np.ascontiguousarray(np.asarray(query, np.float32).reshape(BT, D))
    kv2 = np.ascontiguousarray(np.asarray(key_value, np.float32).reshape(BS, D))
    xT = np.ascontiguousarray(q2.astype(NPBF).T)
    kvT = np.ascontiguousarray(kv2.astype(NPBF).T)
    wq = np.asarray(wq, np.float32)
    wk = np.asarray(wk, np.float32)
    wv = np.asarray(wv, np.float32)
    wo = np.asarray(wo, np.float32)
    in_maps = []
    for c in range(NCORES):
        cs = slice(c * P, (c + 1) * P)
        in_maps.append({
            "xT": xT,
            "kvT": kvT,
            "wqT": np.ascontiguousarray(wq[cs, :].astype(NPBF).T),
            "wkT": np.ascontiguousarray(wk[cs, :].astype(NPBF).T),
            "wvT": np.ascontiguousarray(wv[cs, :].astype(NPBF).T),
            "woT": np.ascontiguousarray(wo[:, cs].astype(NPBF).T),
        })
    return in_maps


def run(inputs, trace=False, **kwargs):
    from concourse.bass_utils import run_bass_kernel_spmd

    nc = _get_nc()
    in_maps = make_in_maps(**inputs)
    res = run_bass_kernel_spmd(
        nc, in_maps, core_ids=list(range(NCORES)), trace=trace, **kwargs
    )
    acc = np.zeros((BT, D), np.float64)
    for r in res.results:
        acc += r["out"].astype(np.float64)
    return acc.astype(np.float32).reshape(B, T, D), res


def kernel(**inputs):
    return run(inputs, trace=False)[0]


# revision 12
# speedup vs baseline: 1.3577x; 1.3577x over previous
"""Trainium2 Bass kernel for nn_CrossAttention (B=4, T=1024, S=2048, D=1024, H=16).

Sharding: tensor-parallel over heads. Each of the 8 cores owns 2 heads
(a 128-column slice of the q/k/v projections and the matching 128-row
slice of the o-projection input). Every core computes a full-shape
partial o-proj output; the host sums the 8 partials (the all-reduce is
done on the host during the gather/unshard step).

Layout strategy: all device matmuls contract along the SBUF partition
axis, so the host pre-transposes the activations and weights (free on
host, removes every on-chip transpose):
  xT  [D, B*T]  = query^T          (bf16)
  kvT [D, B*S]  = key_value^T      (bf16)
  wqT/wkT/wvT [D, 128] per core    (bf16)
  woT [128, D] per core            (bf16)

Pipeline per core (all matmul accumulation in fp32 PSUM):
  qT = WqT.T @ xT            -> [128c, B*T]   (c on partitions)
  kT = WkT.T @ kvT           -> [128c, B*S]
  V  = kvT.T @ WvT           -> [s, c] tiles, stored ones-augmented [128, 65]
  scoresT = kT.T @ qT per head (K=64)         -> [128s, 512t] PSUM
  PT = exp(0.125 * scoresT)  (ACT, no max-subtraction: |scores| < ~7)
  attnT[h] += V_aug.T @ PT   -> [65, 512t]; row 64 = softmax rowsum (free)
  rowsum transposed to [t, 1] via K=1 matmuls, reciprocal on DVE
  o-proj per head (K=64, row-packed) -> [128t, dout] PSUM per head
  out = psum_h0 * (1/r0)[t] + psum_h1 * (1/r1)[t]   (per-partition scalars)
"""

import os
import sys
from contextlib import ExitStack

import numpy as np

for _p in (
    "/root/.axon_site",
    "/root/.axon_site/_ro/trn_rl_repo",
    "/root/.axon_site/_ro/pypackages",
    "/opt/trn_rl_repo",
):
    if os.path.isdir(_p) and _p not in sys.path:
        sys.path.append(_p)

import ml_dtypes  # noqa: E402

import concourse.bass as bass  # noqa: E402
import concourse.mybir as mybir  # noqa: E402
import concourse.tile as tile  # noqa: E402
from concourse import bacc  # noqa: E402

BF = mybir.dt.bfloat16
F32 = mybir.dt.float32
NPBF = ml_dtypes.bfloat16

B, T, S, D = 4, 1024, 2048, 1024
BT, BS = B * T, B * S
P = 128
NCORES = 8
KT = D // P          # 8 contraction tiles of 128
TCH = 512            # free-dim chunk for projections / attention t-chunks
NJ = S // P          # 16 s-tiles of 128 per batch
NST = BS // P        # 64 s-tiles total
EXP_SCALE = float(64 ** -0.5)  # folded into the ACT exp


def build_nc():
    nc = bacc.Bacc("TRN2", target_bir_lowering=False)

    xT = nc.dram_tensor("xT", [D, BT], BF, kind="ExternalInput").ap()
    kvT = nc.dram_tensor("kvT", [D, BS], BF, kind="ExternalInput").ap()
    wqT = nc.dram_tensor("wqT", [D, P], BF, kind="ExternalInput").ap()
    wkT = nc.dram_tensor("wkT", [D, P], BF, kind="ExternalInput").ap()
    wvT = nc.dram_tensor("wvT", [D, P], BF, kind="ExternalInput").ap()
    woT = nc.dram_tensor("woT", [P, D], BF, kind="ExternalInput").ap()
    out = nc.dram_tensor("out", [BT, D], F32, kind="ExternalOutput").ap()

    with tile.TileContext(nc) as tc, ExitStack() as ctx:
        consts = ctx.enter_context(tc.tile_pool(name="consts", bufs=1))
        big = ctx.enter_context(tc.tile_pool(name="big", bufs=1))
        xin = ctx.enter_context(tc.tile_pool(name="xin", bufs=4))
        ptp = ctx.enter_context(tc.tile_pool(name="ptp", bufs=3))
        atsb = ctx.enter_context(tc.tile_pool(name="atsb", bufs=2))
        smalls = ctx.enter_context(tc.tile_pool(name="smalls", bufs=4))
        outp = ctx.enter_context(tc.tile_pool(name="outp", bufs=3))
        # PSUM budget (8 banks): mm [128,1024]x2 = 4 + at 2 + op 2
        mm_ps = ctx.enter_context(tc.tile_pool(name="mm_ps", bufs=2, space="PSUM"))
        at_pool = ctx.enter_context(tc.tile_pool(name="at_ps", bufs=2, space="PSUM"))
        op_pool = ctx.enter_context(tc.tile_pool(name="op_ps", bufs=2, space="PSUM"))

        # ---- resident weights ----
        wq_s = consts.tile([P, KT, P], BF, tag="wq_s")
        wk_s = consts.tile([P, KT, P], BF, tag="wk_s")
        wv_s = consts.tile([P, KT, P], BF, tag="wv_s")
        wqT_t = wqT.rearrange("(kt p) c -> p kt c", p=P)
        wkT_t = wkT.rearrange("(kt p) c -> p kt c", p=P)
        wvT_t = wvT.rearrange("(kt p) c -> p kt c", p=P)
        for kt in range(KT):
            nc.sync.dma_start(wq_s[:, kt, :], wqT_t[:, kt, :])
            nc.sync.dma_start(wk_s[:, kt, :], wkT_t[:, kt, :])
            nc.sync.dma_start(wv_s[:, kt, :], wvT_t[:, kt, :])
        wo_s = consts.tile([P, D], BF, tag="wo_s")
        nc.sync.dma_start(wo_s[:], woT)
        ones1 = consts.tile([1, 1], F32, tag="ones1")
        nc.gpsimd.memset(ones1[:], 1.0)

        # ---- resident intermediates ----
        qT_s = big.tile([P, BT], BF, tag="qT_s")
        kT_s = big.tile([P, BS], BF, tag="kT_s")
        # Per-head V, ones-augmented: 64 s-tiles, each [128, 65] with col 64 == 1.0
        v_s = [
            big.tile([P, NST * 65], BF, tag=f"v{h}_s", name=f"v{h}_s")
            for h in range(2)
        ]
        for h in range(2):
            nc.gpsimd.memset(v_s[h][:], 1.0)

        xT_t = xT.rearrange("(kt p) t -> p kt t", p=P)
        kvT_t = kvT.rearrange("(kt p) s -> p kt s", p=P)

        def q_proj_chunk(ch):
            x_t = xin.tile([P, KT, TCH], BF, tag="in_t", name="x_t")
            for kt in range(KT):
                nc.sync.dma_start(
                    x_t[:, kt, :], xT_t[:, kt, ch * TCH:(ch + 1) * TCH]
                )
            ps = mm_ps.tile([P, 1024], F32, tag="mm", name="qps")
            for kt in range(KT):
                nc.tensor.matmul(
                    ps[:, :TCH], wq_s[:, kt, :], x_t[:, kt, :],
                    start=(kt == 0), stop=(kt == KT - 1),
                )
            nc.vector.tensor_copy(qT_s[:, ch * TCH:(ch + 1) * TCH], ps[:, :TCH])

        def kv_proj_chunk(ch):
            kv_t = xin.tile([P, KT, TCH], BF, tag="in_t", name="kv_t")
            for kt in range(KT):
                nc.sync.dma_start(
                    kv_t[:, kt, :], kvT_t[:, kt, ch * TCH:(ch + 1) * TCH]
                )
            ps = mm_ps.tile([P, 1024], F32, tag="mm", name="kps")
            for kt in range(KT):
                nc.tensor.matmul(
                    ps[:, :TCH], wk_s[:, kt, :], kv_t[:, kt, :],
                    start=(kt == 0), stop=(kt == KT - 1),
                )
            nc.vector.tensor_copy(kT_s[:, ch * TCH:(ch + 1) * TCH], ps[:, :TCH])
            # V projection: [s, c] orientation, 4 s-subtiles share one bank.
            # start only on the bank's first matmul: start=True marks the
            # whole 2KB zero-region pending-zero, so later subtiles' first
            # writes overwrite (not accumulate) stale data automatically.
            vps = mm_ps.tile([P, 4, P], F32, tag="mm", name="vps")
            for kt in range(KT):
                for sub in range(4):
                    nc.tensor.matmul(
                        vps[:, sub, :],
                        kv_t[:, kt, sub * P:(sub + 1) * P],
                        wv_s[:, kt, :],
                        start=(kt == 0 and sub == 0),
                        stop=(kt == KT - 1 and sub == 3),
                    )
            for sub in range(4):
                jg = ch * 4 + sub
                nc.vector.tensor_copy(
                    v_s[0][:, jg * 65:jg * 65 + 64], vps[:, sub, 0:64]
                )
                nc.vector.tensor_copy(
                    v_s[1][:, jg * 65:jg * 65 + 64], vps[:, sub, 64:128]
                )

        def attention_block(b, t2):
            t0 = b * T + t2 * TCH
            ats = [
                at_pool.tile([65, TCH], F32, tag="at", name=f"at{h}")
                for h in range(2)
            ]
            for j in range(NJ):
                jg = b * NJ + j
                sc = mm_ps.tile([P, 1024], F32, tag="mm", name="sc")
                for h in range(2):
                    hp = h * 64
                    nc.tensor.matmul(
                        sc[:, h * TCH:(h + 1) * TCH],
                        kT_s[hp:hp + 64, b * S + j * P: b * S + (j + 1) * P],
                        qT_s[hp:hp + 64, t0:t0 + TCH],
                        start=True, stop=True,
                    )
                pt = ptp.tile([P, 1024], BF, tag="pt", name="pt")
                nc.scalar.activation(
                    pt[:], sc[:],
                    mybir.ActivationFunctionType.Exp,
                    scale=EXP_SCALE,
                )
                for h in range(2):
                    nc.tensor.matmul(
                        ats[h][:],
                        v_s[h][:, jg * 65:(jg + 1) * 65],
                        pt[:, h * TCH:(h + 1) * TCH],
                        start=(j == 0), stop=(j == NJ - 1),
                    )

            # rowsums -> [t, 1] layout via K=1 matmuls, then reciprocal
            rt_ps = mm_ps.tile([P, 1024], F32, tag="mm", name="rt_ps")
            aT = atsb.tile([P, TCH], BF, tag="aT", name="aT")
            for h in range(2):
                r_sb = smalls.tile([1, TCH], F32, tag="rsb", name="r_sb")
                nc.vector.tensor_copy(r_sb[:], ats[h][64:65, :])
                for sub in range(4):
                    nc.tensor.matmul(
                        rt_ps[:, h * 4 + sub:h * 4 + sub + 1],
                        r_sb[0:1, sub * P:(sub + 1) * P],
                        ones1[0:1, 0:1],
                        start=True, stop=True,
                    )
                nc.vector.tensor_copy(aT[h * 64:(h + 1) * 64, :], ats[h][0:64, :])
            rt = smalls.tile([P, 8], F32, tag="rtr", name="rt")
            nc.vector.reciprocal(rt[:], rt_ps[:, :8])

            # o-proj per head (K=64, row-packed pair); fused combine with 1/r
            for sub in range(4):
                ot = outp.tile([P, D], F32, tag="ot", name="ot")
                for n in range(D // TCH):  # 2
                    op0 = op_pool.tile([P, TCH], F32, tag="op", name="op0")
                    op1 = op_pool.tile([P, TCH], F32, tag="op", name="op1")
                    nc.tensor.matmul(
                        op0[:],
                        aT[0:64, sub * P:(sub + 1) * P],
                        wo_s[0:64, n * TCH:(n + 1) * TCH],
                        start=True, stop=True,
                    )
                    nc.tensor.matmul(
                        op1[:],
                        aT[64:128, sub * P:(sub + 1) * P],
                        wo_s[64:128, n * TCH:(n + 1) * TCH],
                        start=True, stop=True,
                    )
                    osl = ot[:, n * TCH:(n + 1) * TCH]
                    nc.vector.tensor_scalar_mul(osl, op1[:], rt[:, 4 + sub:5 + sub])
                    nc.vector.scalar_tensor_tensor(
                        osl, op0[:], rt[:, sub:sub + 1], osl,
                        mybir.AluOpType.mult, mybir.AluOpType.add,
                    )
                nc.sync.dma_start(out[t0 + sub * P:t0 + (sub + 1) * P, :], ot[:])

        # b-major interleave: project batch b's q/k/v, then attend, so PE
        # stays dense and batch b+1's DMA overlaps batch b's attention.
        for b in range(B):
            q_proj_chunk(2 * b)
            q_proj_chunk(2 * b + 1)
            for ch in range(4 * b, 4 * b + 4):
                kv_proj_chunk(ch)
            attention_block(b, 0)
            attention_block(b, 1)

    nc.compile()
    return nc


_NC_CACHE = None


def _get_nc():
    global _NC_CACHE
    if _NC_CACHE is None:
        _NC_CACHE = build_nc()
    return _NC_CACHE


def make_in_maps(query, key_value, wq, wk, wv, wo):
    q2 = np.ascontiguousarray(np.asarray(query, np.float32).reshape(BT, D))
    kv2 = np.ascontiguousarray(np.asarray(key_value, np.float32).reshape(BS, D))
    xT = np.ascontiguousarray(q2.astype(NPBF).T)
    kvT = np.ascontiguousarray(kv2.astype(NPBF).T)
    wq = np.asarray(wq, np.float32)
    wk = np.asarray(wk, np.float32)
    wv = np.asarray(wv, np.float32)
    wo = np.asarray(wo, np.float32)
    in_maps = []
    for c in range(NCORES):
        cs = slice(c * P, (c + 1) * P)
        in_maps.append({
            "xT": xT,
            "kvT": kvT,
            "wqT": np.ascontiguousarray(wq[cs, :].astype(NPBF).T),
            "wkT": np.ascontiguousarray(wk[cs, :].astype(NPBF).T),
            "wvT": np.ascontiguousarray(wv[cs, :].astype(NPBF).T),
            "woT": np.ascontiguousarray(wo[:, cs].astype(NPBF).T),
        })
    return in_maps


def run(inputs, trace=False, **kwargs):
    from concourse.bass_utils import run_bass_kernel_spmd

    nc = _get_nc()
    in_maps = make_in_maps(**inputs)
    res = run_bass_kernel_spmd(
        nc, in_maps, core_ids=list(range(NCORES)), trace=trace, **kwargs
    )
    acc = np.zeros((BT, D), np.float64)
    for r in res.results:
        acc += r["out"].astype(np.float64)
    return acc.astype(np.float32).reshape(B, T, D), res


def kernel(**inputs):
    return run(inputs, trace=False)[0]
